# revision 64
# baseline (speedup 1.0000x reference)
"""Causal multi-head attention (B=4, T=2048, C=1024, H=16) on 8 Trainium2 cores.

Sharding: core c handles batch b = c//2 and heads h0..h0+7 with h0 = (c%2)*8.
Each core computes QKV projection for its head slice, causal attention for its
8 heads, and a partial output projection. Host sums the two partials per batch
and adds the bias terms.

Row-blocked mixed precision (host-validated to ~8e-3 max-rel, gate 2e-2):
 - Query rows 0-511 ("strip 0") run bf16: early rows attend to few keys so
   attn ~ a single v row with sigma ~1, leaving them ~30x less absolute-error
   headroom than late rows (sigma ~0.1).
 - Rows 512-2047 (strips 1-3) run fp8: QKV projection and output projection
   as float8e4 DoubleRow matmuls (0.5 PE cycles/row, 256-deep contraction),
   and AV as DoubleRow over k-tile pairs. Softmax probs u are fp8: e4m3 from
   ScalarE exp on diagonal tiles, e5m2 via a Schraudolph exp on DVE for full
   tiles (u8 = round(A*s + B) bitcast as e5m2), which offloads about half the
   exp work from the Activation engine.
 - AV runs TRANSPOSED: u (stationary, fp8, M=128 q-slice) x v (moving, 65
   cols incl. the ones column) -> attn^T [q, hd] chunks accumulated per
   q-subtile. The softmax rowsum lands in column 64 (per-partition), so
   normalization is reciprocal + one tensor_scalar per q-subtile - no
   cross-partition broadcast. A PE transpose (bf16) then restores the [c, t]
   layout needed by the projection. DoubleRow ldweights requires stationary
   free dim 128/256 (65-wide stationary is rejected by the ISA), which this
   layout sidesteps.
 - Weights host-prescaled x32 (fp8 subnormal avoidance); ones column 4.0 so
   attnT holds 8*attn; the y copy scales by 1/256. k's bias is dropped
   (softmax-invariant); q keeps its bias. A Delta=2 shift inside exp
   (softmax-invariant) keeps exp(s) in e4m3 range.

Scores stay bf16 ([k, q] layout; causal mask as a -1e30 bf16 ident-matmul
into PSUM before the exp; per-128-column causal trimming).
"""

import sys
import numpy as np

sys.path.insert(0, "/opt/trn_rl_repo")

import concourse.bass as bass  # noqa: E402
import concourse.bacc as bacc  # noqa: E402
import concourse.mybir as mybir  # noqa: E402
from concourse.bass_utils import run_bass_kernel_spmd  # noqa: E402
from concourse.tile import TileContext  # noqa: E402

B, T, C, H = 4, 2048, 1024, 16
HD = C // H          # 64 head dim
HPC = 8              # heads per core
P = 128
NT = T // P          # 16 t-chunks of 128
NS = T // 512        # 4 q-strips of 512
KC = C // P          # 8 contraction chunks for QKV
CL = HPC * HD        # 512 local channels per section
EH = HD + 1          # 65: head slot width in v (value cols + ones col)
T0 = 512             # bf16/fp8 row boundary
F32 = mybir.dt.float32
BF16 = mybir.dt.bfloat16
E4 = mybir.dt.float8e4
E5 = mybir.dt.float8e5
U8 = mybir.dt.uint8
EXPF = mybir.ActivationFunctionType.Exp
IDENT = mybir.ActivationFunctionType.Identity
MUL = mybir.AluOpType.mult
ADD = mybir.AluOpType.add
DIV = mybir.AluOpType.divide
DRM = mybir.MatmulPerfMode.DoubleRow

WSC = 32.0           # host weight prescale
ONESV = 4.0          # ones-column value -> attnT = 8*attn
YSC = 1.0 / 256.0    # final y scale (1/(8*32))
SC = 0.125 / (WSC * WSC)   # exp scale on raw PSUM scores
DELTA = 2.0          # softmax-invariant shift inside exp
SCH_A = float((4.0 / np.log(2.0)) * SC)
SCH_B = float(4.0 * 15 - 0.23 - (4.0 / np.log(2.0)) * DELTA)

_CACHED = {}


def build_nc():
    nc = bacc.Bacc("TRN2", target_bir_lowering=False, debug=False)

    xtb_d = nc.dram_tensor("xtb", [C, T0], BF16, kind="ExternalInput")
    xt8_d = nc.dram_tensor("xt8", [C, T - T0], E4, kind="ExternalInput")
    wqkb_d = nc.dram_tensor("wqkb", [C, 2 * CL], BF16, kind="ExternalInput")
    wqk8_d = nc.dram_tensor("wqk8", [C, 2 * CL], E4, kind="ExternalInput")
    wvb_d = nc.dram_tensor("wvb", [C, CL], BF16, kind="ExternalInput")
    wv8_d = nc.dram_tensor("wv8", [C, CL], E4, kind="ExternalInput")
    wpb_d = nc.dram_tensor("wpb", [CL, C], BF16, kind="ExternalInput")
    wp8_d = nc.dram_tensor("wp8", [CL, C], E4, kind="ExternalInput")
    bq_d = nc.dram_tensor("bq", [P, 4], F32, kind="ExternalInput")
    ident_d = nc.dram_tensor("ident", [P, P], BF16, kind="ExternalInput")
    maskb_d = nc.dram_tensor("maskb", [P, P], BF16, kind="ExternalInput")
    y_d = nc.dram_tensor("y", [T, C], BF16, kind="ExternalOutput")

    xtb_r = xtb_d.ap().rearrange("(kc p) t -> p kc t", p=P)     # [128,8,512]
    xt8_r = xt8_d.ap().rearrange("(kc p) t -> p kc t", p=P)     # [128,8,1536]
    wqkb_r = wqkb_d.ap().rearrange("(kc p) c -> p kc c", p=P)   # [128,8,1024]
    wqk8_r = wqk8_d.ap().rearrange("(kc p) c -> p kc c", p=P)
    wvb_r = wvb_d.ap().rearrange("(kc p) c -> p kc c", p=P)     # [128,8,512]
    wv8_r = wv8_d.ap().rearrange("(kc p) c -> p kc c", p=P)
    wpb_r = wpb_d.ap().rearrange("(ct p) c -> p ct c", p=P)     # [128,4,1024]
    wp8_r = wp8_d.ap().rearrange("(ct p) c -> p ct c", p=P)
    y_r = y_d.ap().rearrange("(tt p) c -> p tt c", p=P)         # [128,16,1024]

    SW = 256             # phase-A t-strip width
    NB = T0 // SW        # 2 bf16 strips
    NTB = T0 // P        # 4 bf16 t-chunks

    with TileContext(nc) as tc:
      with tc.tile_pool(name="const", bufs=1) as constp:
        ident = constp.tile([P, P], BF16)
        maskb = constp.tile([P, P], BF16)
        bq = constp.tile([P, 4], F32)
        bias_m2 = constp.tile([P, 1], F32)
        nc.vector.memset(bias_m2[:], -DELTA)

        with tc.tile_pool(name="big", bufs=1) as bigp:
            qkT = bigp.tile([P, 8, T], BF16)  # c-tiles 0-3 qT (biased), 4-7 kT
            v_bf = bigp.tile([P, NTB, HPC * EH], BF16)
            v_f8 = bigp.tile([P, NT, HPC * EH], E4)
            vbf_h = v_bf[:].rearrange("p t (h e) -> p t h e", e=EH)
            vf8_h = v_f8[:].rearrange("p t (h e) -> p t h e", e=EH)
            nc.gpsimd.memset(vbf_h[:, :, :, HD], ONESV)
            nc.gpsimd.memset(vf8_h[:, :, :, HD], ONESV)
            attnT_bf = bigp.tile([P, 4, T0], BF16)
            attnT_f8 = bigp.tile([P, 4, T - T0], E4)
            wpb_sb = bigp.tile([P, 4, C], BF16)
            wp8_sb = bigp.tile([P, 4, C], E4)

            # ------- Phase A (QKV) with strip-0 attention overlapped -------
            # The fp8 projection strips (ts 2-7) are PE/DMA work with little
            # Act/DVE load, while strip-0 attention is exp/copy-heavy with
            # little PE work: interleaving their emission fills both. Strip-0
            # only needs ts 0-1 (q,k,v rows < 512), so its pairs slot in
            # between the later projection strips. PSUM during the overlap:
            # psa 1 + psq 2 + ring 2 + po 2 + psT 1 = 8 banks.
            with (
                tc.tile_pool(name="u_act", bufs=16) as uap,
                tc.tile_pool(name="u_dve", bufs=12) as udp,
                tc.tile_pool(name="u_bf", bufs=10) as u0p,
                tc.tile_pool(name="an_p", bufs=10) as anp,
            ):
                expctr = [0]

                def emit_scores_exp(qj, pr, ps_s):
                    """Scores + exp for one (strip, pair). Returns the list
                    of u tiles (one per kt-pair for qj>0, per kt for qj==0)
                    for the AV stage."""
                    qct, kct = pr, 4 + pr
                    qlo, qhi = qj * 512, (qj + 1) * 512
                    us = []
                    if qj == 0:
                        for kt in range(4):
                            q0 = kt * P
                            u = u0p.tile([P, 2, 512], BF16, tag="u0")
                            for hh in range(2):
                                hp = hh * HD
                                ps = ps_s.tile([P, 512], F32, tag="ps")
                                nc.tensor.matmul(
                                    ps[:, q0:512],
                                    qkT[hp:hp + HD, kct,
                                        kt * P:(kt + 1) * P],
                                    qkT[hp:hp + HD, qct, q0:512],
                                    start=True, stop=False,
                                )
                                nc.tensor.matmul(
                                    ps[:, q0:q0 + P], ident[:], maskb[:],
                                    start=False, stop=True,
                                )
                                nc.scalar.activation(
                                    u[:, hh, q0:512], ps[:, q0:512],
                                    EXPF, scale=SC, bias=bias_m2[:])
                            us.append(u)
                        return us
                    npair = 2 * (qj + 1)
                    for ktp in range(npair):
                        ktA, ktB = 2 * ktp, 2 * ktp + 1
                        diag = ktA >= 4 * qj
                        q0A = max(0, ktA * P - qj * 512)
                        q0B = max(0, ktB * P - qj * 512)
                        # each ktp's 4 exp units split across BOTH engines
                        # (hh0 -> Act native exp, hh1 -> DVE schraudolph,
                        # both writing e5m2) so neither engine idles while
                        # the other works. Diagonal tiles stay all-Act
                        # (-1e30 masked scores are unsafe through the u8
                        # bitcast).
                        u = uap.tile([P, 2, 2, 512], E5, tag="u5a")
                        for kb, (kt, q0) in enumerate(
                                ((ktA, q0A), (ktB, q0B))):
                            for hh in range(2):
                                hp = hh * HD
                                ps = ps_s.tile([P, 512], F32, tag="ps")
                                nc.tensor.matmul(
                                    ps[:, q0:512],
                                    qkT[hp:hp + HD, kct,
                                        kt * P:(kt + 1) * P],
                                    qkT[hp:hp + HD, qct,
                                        qlo + q0:qhi],
                                    start=True, stop=not diag,
                                )
                                if diag:
                                    nc.tensor.matmul(
                                        ps[:, q0:q0 + P], ident[:], maskb[:],
                                        start=False, stop=True,
                                    )
                                if diag or hh == 0:
                                    nc.scalar.activation(
                                        u[:, kb, hh, q0:512], ps[:, q0:512],
                                        EXPF, scale=SC, bias=bias_m2[:])
                                else:
                                    nc.vector.tensor_scalar(
                                        u[:, kb, hh, q0:512].bitcast(U8),
                                        ps[:, q0:512],
                                        SCH_A, SCH_B, MUL, ADD)
                        if diag and q0B > q0A:
                            # block B's columns [q0A:q0B] are fully masked
                            # but get read by the q0A-subtile AV chain
                            nc.gpsimd.memset(u[:, 1, :, q0A:q0B], 0.0)
                        if not diag:
                            expctr[0] += 1
                        us.append(u)
                    return us

                def emit_av_norm(qj, pr, us, ps_o, ps_t):
                    """Transposed AV + normalize + transpose into attnT."""
                    psT = ps_t.tile([P, 1024], BF16, tag="psT")
                    for qt in range(4):
                        poQ = {}
                        for hh in range(2):
                            h = 2 * pr + hh
                            po = ps_o.tile([P, 512], F32, tag="po")
                            poQ[hh] = po
                            if qj == 0:
                                chain = [(kt, us[kt]) for kt in range(qt + 1)]
                                for i, (kt, u) in enumerate(chain):
                                    nc.tensor.matmul(
                                        po[:, 0:EH],
                                        u[:, hh, qt * P:(qt + 1) * P],
                                        vbf_h[:, kt, h, :],
                                        start=(i == 0),
                                        stop=(i == len(chain) - 1),
                                    )
                            else:
                                chain = []
                                for ktp, u in enumerate(us):
                                    q0A = max(0, 2 * ktp * P - qj * 512)
                                    if q0A <= qt * P:
                                        chain.append((ktp, u))
                                for i, (ktp, u) in enumerate(chain):
                                    nc.tensor.matmul(
                                        po[:, 0:EH],
                                        u[:, :, hh, qt * P:(qt + 1) * P],
                                        vf8_h[:, 2 * ktp:2 * ktp + 2, h, :],
                                        start=(i == 0),
                                        stop=(i == len(chain) - 1),
                                        perf_mode=DRM,
                                    )
                        for hh in range(2):
                            po = poQ[hh]
                            rcol = anp.tile([P, 1], F32, tag="rc")
                            nc.vector.reciprocal(rcol[:], po[:, HD:EH])
                            a_n = anp.tile([P, HD], BF16, tag="an")
                            nc.vector.tensor_scalar(
                                a_n[:], po[:, 0:HD], rcol[:], None, MUL)
                            # 8 transposes share one PSUM bank: open the
                            # zero region once per partition half (qt==0),
                            # close on the last qt - a fresh start=True per
                            # transpose would re-mark the whole 2KB region
                            # pending-zero and wipe the sibling writes.
                            nc.tensor.matmul(
                                psT[hh * HD:(hh + 1) * HD,
                                    qt * P:(qt + 1) * P],
                                a_n[:], ident[:], is_transpose=True,
                                start=(qt == 0), stop=(qt == 3),
                            )
                    if qj == 0:
                        nc.scalar.copy(
                            attnT_bf[:, pr, qj * 512:(qj + 1) * 512],
                            psT[:, 0:512])
                    else:
                        nc.vector.tensor_copy(
                            attnT_f8[:, pr, qj * 512 - T0:(qj + 1) * 512 - T0],
                            psT[:, 0:512])

                with (
                    tc.tile_pool(name="xts", bufs=4) as xtsp,
                    tc.tile_pool(name="w_pool", bufs=1) as wqp,
                    tc.tile_pool(name="ps_av", bufs=1, space="PSUM") as ps_av,
                    tc.tile_pool(name="ps_aq", bufs=2, space="PSUM") as ps_aq,
                    tc.tile_pool(name="ps_s1", bufs=2, space="PSUM") as ps_s1,
                    tc.tile_pool(name="ps_o1", bufs=2, space="PSUM") as ps_o1,
                    tc.tile_pool(name="ps_t1", bufs=1, space="PSUM") as ps_t1,
                ):
                    wvb_sb = wqp.tile([P, KC, CL], BF16)
                    wv8_sb = wqp.tile([P, KC, CL], E4)
                    wqkb_sb = wqp.tile([P, KC, 2 * CL], BF16)
                    wqk8_sb = wqp.tile([P, KC, 2 * CL], E4)
                    xts0 = xtsp.tile([P, KC, SW], BF16, tag="xtsb")
                    nc.sync.dma_start(xts0[:, 0, :], xtb_r[:, 0, 0:SW])
                    nc.sync.dma_start(wvb_sb[:, 0, :], wvb_r[:, 0, :])
                    nc.sync.dma_start(ident[:], ident_d[:])
                    nc.sync.dma_start(maskb[:], maskb_d[:])
                    nc.sync.dma_start(bq[:], bq_d[:])
                    nc.sync.dma_start(xts0[:, 1:KC, :], xtb_r[:, 1:KC, 0:SW])
                    nc.sync.dma_start(wvb_sb[:, 1:KC, :], wvb_r[:, 1:KC, :])
                    nc.sync.dma_start(wqkb_sb[:], wqkb_r)

                    # prefetch-next x-strip: strip ts+1's DMA is issued ahead
                    # of deferred weight loads so it never queues behind them
                    def xts_dma(ts):
                        if ts >= T // SW:
                            return None
                        if ts < NB:
                            t = xtsp.tile([P, KC, SW], BF16, tag="xtsb",
                                          name=f"xts_{ts}")
                            nc.sync.dma_start(
                                t[:], xtb_r[:, :, ts * SW:(ts + 1) * SW])
                        else:
                            t = xtsp.tile([P, KC, SW], E4, tag="xts8",
                                          name=f"xts_{ts}")
                            nc.sync.dma_start(
                                t[:],
                                xt8_r[:, :, ts * SW - T0:(ts + 1) * SW - T0])
                        return t

                    pf = {1: xts_dma(1)}
                    nc.sync.dma_start(wv8_sb[:], wv8_r)
                    nc.sync.dma_start(wqk8_sb[:], wqk8_r)

                    def emit_ts(ts):
                        is_bf = ts < NB
                        xts = xts0 if ts == 0 else pf.pop(ts)
                        pf[ts + 1] = xts_dma(ts + 1)
                        if ts == 5:
                            # proj weights aren't needed until phase C
                            # (~170us); keep them out of the DMA stream
                            # while the input tensors are still draining
                            nc.sync.dma_start(wpb_sb[:], wpb_r)
                            nc.sync.dma_start(wp8_sb[:], wp8_r)
                        # v part: [t, c] orientation
                        for tt in range(SW // P):
                            tch = ts * (SW // P) + tt
                            psv = ps_av.tile([P, CL], F32, tag="psa")
                            if is_bf:
                                for kc in range(KC):
                                    nc.tensor.matmul(
                                        psv[:],
                                        xts[:, kc, tt * P:(tt + 1) * P],
                                        wvb_sb[:, kc, :],
                                        start=(kc == 0), stop=(kc == KC - 1),
                                    )
                                nc.vector.tensor_copy(
                                    vbf_h[:, tch, :, 0:HD],
                                    psv[:].rearrange("p (h d) -> p h d", d=HD),
                                )
                            else:
                                for kc2 in range(KC // 2):
                                    nc.tensor.matmul(
                                        psv[:],
                                        xts[:, 2 * kc2:2 * kc2 + 2,
                                            tt * P:(tt + 1) * P],
                                        wv8_sb[:, 2 * kc2:2 * kc2 + 2, :],
                                        start=(kc2 == 0),
                                        stop=(kc2 == KC // 2 - 1),
                                        perf_mode=DRM,
                                    )
                            nc.vector.tensor_copy(
                                vf8_h[:, tch, :, 0:HD],
                                psv[:].rearrange("p (h d) -> p h d", d=HD),
                            )
                        # qT/kT part: [c, t] orientation; bf16 strips emit
                        # c-tiles in head-pair order (q0,k0,q1,k1,...) so
                        # strip-0 pair pr can score as soon as its two
                        # tiles land
                        cts = [0, 4, 1, 5, 2, 6, 3, 7] if is_bf else range(8)
                        for ct in cts:
                            psq = ps_aq.tile([P, SW], F32, tag="psq")
                            if is_bf:
                                for kc in range(KC):
                                    nc.tensor.matmul(
                                        psq[:],
                                        wqkb_sb[:, kc, ct * P:(ct + 1) * P],
                                        xts[:, kc, :],
                                        start=(kc == 0), stop=(kc == KC - 1),
                                    )
                            else:
                                for kc2 in range(KC // 2):
                                    nc.tensor.matmul(
                                        psq[:],
                                        wqk8_sb[:, 2 * kc2:2 * kc2 + 2,
                                                ct * P:(ct + 1) * P],
                                        xts[:, 2 * kc2:2 * kc2 + 2, :],
                                        start=(kc2 == 0),
                                        stop=(kc2 == KC // 2 - 1),
                                        perf_mode=DRM,
                                    )
                            dst = qkT[:, ct, ts * SW:(ts + 1) * SW]
                            if ct < 4 and ct % 2 == 0:
                                nc.scalar.activation(
                                    dst, psq[:], IDENT, bias=bq[:, ct:ct + 1])
                            elif ct < 4:
                                nc.vector.tensor_scalar(
                                    dst, psq[:], bq[:, ct:ct + 1], None, ADD)
                            elif ct % 2 == 0:
                                nc.vector.tensor_copy(dst, psq[:])
                            else:
                                nc.scalar.copy(dst, psq[:])

                    emit_ts(0)
                    emit_ts(1)
                    s0us = []
                    s0us.append(emit_scores_exp(0, 0, ps_s1))
                    emit_ts(2)
                    emit_ts(3)
                    s0us.append(emit_scores_exp(0, 1, ps_s1))
                    emit_av_norm(0, 0, s0us[0], ps_o1, ps_t1)
                    emit_ts(4)
                    s0us.append(emit_scores_exp(0, 2, ps_s1))
                    emit_av_norm(0, 1, s0us[1], ps_o1, ps_t1)
                    emit_ts(5)
                    s0us.append(emit_scores_exp(0, 3, ps_s1))
                    emit_av_norm(0, 2, s0us[2], ps_o1, ps_t1)
                    emit_ts(6)
                    emit_ts(7)
                    # strip-1 pr0's scores ride the overlap ring too, so the
                    # big-ring scope below starts with an AV already fed
                    s1u0 = emit_scores_exp(1, 0, ps_s1)
                    emit_av_norm(0, 3, s0us[3], ps_o1, ps_t1)

                # ---- strips 1-3: full-depth scores ring ----
                with (
                    tc.tile_pool(name="ps_s2", bufs=5, space="PSUM") as ps_s2,
                    tc.tile_pool(name="ps_o2", bufs=2, space="PSUM") as ps_o2,
                    tc.tile_pool(name="ps_t2", bufs=1, space="PSUM") as ps_t2,
                ):
                    prev = (1, 0, s1u0, ps_o2, ps_t2)
                    for qj in range(1, NS):
                        for pr in range(4):
                            if qj == 1 and pr == 0:
                                continue
                            us = emit_scores_exp(qj, pr, ps_s2)
                            if prev is not None:
                                emit_av_norm(*prev)
                            prev = (qj, pr, us, ps_o2, ps_t2)
                    if prev is not None:
                        emit_av_norm(*prev)

            # ---------------- Phase C: output projection ----------------
            with (
                tc.tile_pool(name="ystage", bufs=8) as ystagep,
                tc.tile_pool(name="ps_f", bufs=4, space="PSUM") as ps_f,
            ):
                i = 0
                # fp8 chains (tt>=4) are 4x shorter on the PE than the bf16
                # ones; lead with a few so the copy/DMA drain starts early.
                # Both co-halves of a tt share one 2-bank psy tile (halves
                # are 2KB-aligned, so their accumulation groups don't clash)
                # -> one [128,1024] copy + DMA per tt instead of two.
                ttorder = [4, 5, 0, 6, 7, 1, 8, 9, 2, 10, 11, 3,
                           12, 13, 14, 15]
                for tt in ttorder:
                    psy = ps_f.tile([P, 1024], F32, tag="psf",
                                    name=f"psf_{tt}")
                    for co in range(2):
                        if tt < 4:
                            for ct in range(4):
                                nc.tensor.matmul(
                                    psy[:, co * 512:(co + 1) * 512],
                                    attnT_bf[:, ct, tt * P:(tt + 1) * P],
                                    wpb_sb[:, ct, co * 512:(co + 1) * 512],
                                    start=(ct == 0), stop=(ct == 3),
                                )
                        else:
                            for cp in range(2):
                                nc.tensor.matmul(
                                    psy[:, co * 512:(co + 1) * 512],
                                    attnT_f8[:, 2 * cp:2 * cp + 2,
                                             (tt - 4) * P:(tt - 3) * P],
                                    wp8_sb[:, 2 * cp:2 * cp + 2,
                                           co * 512:(co + 1) * 512],
                                    start=(cp == 0), stop=(cp == 1),
                                    perf_mode=DRM,
                                )
                    yt = ystagep.tile([P, 1024], BF16, tag="yt")
                    if i % 2 == 0:
                        nc.scalar.mul(yt[:], psy[:], YSC)
                    else:
                        nc.vector.tensor_scalar(
                            yt[:], psy[:], YSC, None, MUL)
                    nc.sync.dma_start(y_r[:, tt, :], yt[:])
                    i += 1
    nc.compile()
    return nc


def _host_consts():
    import ml_dtypes
    i_idx = np.arange(P, dtype=np.float32)[:, None]
    j_idx = np.arange(P, dtype=np.float32)[None, :]
    maskb = np.where(j_idx - i_idx >= 0, 0.0, -1e30).astype(ml_dtypes.bfloat16)
    ident = np.eye(P, dtype=ml_dtypes.bfloat16)
    return ident, maskb


def make_in_maps(x, w_attn, b_attn, w_proj):
    import ml_dtypes
    ident, maskb = _host_consts()
    in_maps = []
    for c in range(8):
        b = c // 2
        h0 = (c % 2) * HPC
        qcols = slice(h0 * HD, h0 * HD + CL)
        kcols = slice(C + h0 * HD, C + h0 * HD + CL)
        vcols = slice(2 * C + h0 * HD, 2 * C + h0 * HD + CL)
        wqk = np.concatenate([w_attn[:, qcols], w_attn[:, kcols]],
                             axis=1) * WSC
        wv = w_attn[:, vcols] * WSC
        wp = w_proj[h0 * HD:h0 * HD + CL, :] * WSC
        bqv = (b_attn[qcols] * WSC).reshape(4, P).T  # [128, 4] q bias only
        xt = np.ascontiguousarray(x[b].T)
        in_maps.append({
            "xtb": xt[:, :T0].astype(ml_dtypes.bfloat16),
            "xt8": xt[:, T0:].astype(ml_dtypes.float8_e4m3),
            "wqkb": wqk.astype(ml_dtypes.bfloat16),
            "wqk8": wqk.astype(ml_dtypes.float8_e4m3),
            "wvb": wv.astype(ml_dtypes.bfloat16),
            "wv8": wv.astype(ml_dtypes.float8_e4m3),
            "wpb": wp.astype(ml_dtypes.bfloat16),
            "wp8": wp.astype(ml_dtypes.float8_e4m3),
            "bq": np.ascontiguousarray(bqv).astype(np.float32),
            "ident": ident,
            "maskb": maskb,
        })
    return in_maps


def _get_runner():
    """Build the SPMD executor once: a cached jax.jit over 8 cores."""
    if "runner" in _CACHED:
        return _CACHED["runner"]
    import jax
    from jax.sharding import Mesh, PartitionSpec
    from jax.experimental.shard_map import shard_map
    from concourse import bass2jax
    import concourse.mybir as mybir_

    nc = _CACHED.get("nc")
    if nc is None:
        nc = _CACHED["nc"] = build_nc()
    bass2jax.install_neuronx_cc_hook()

    partition_name = (nc.partition_id_tensor.name
                      if nc.partition_id_tensor else None)
    in_names, out_names, out_avals, zero_shapes = [], [], [], []
    for alloc in nc.m.functions[0].allocations:
        if not isinstance(alloc, mybir_.MemoryLocationSet):
            continue
        name = alloc.memorylocations[0].name
        if alloc.kind == "ExternalInput":
            if name != partition_name:
                in_names.append(name)
        elif alloc.kind == "ExternalOutput":
            shape = tuple(alloc.tensor_shape)
            dtype = mybir_.dt.np(alloc.dtype)
            out_names.append(name)
            out_avals.append(jax.core.ShapedArray(shape, dtype))
            zero_shapes.append((shape, dtype))
    n_params = len(in_names)
    n_outs = len(out_names)
    all_names = in_names + out_names
    if partition_name is not None:
        all_names = all_names + [partition_name]

    def _body(*args):
        operands = list(args)
        if partition_name is not None:
            operands.append(bass2jax.partition_id_tensor())
        outs = bass2jax._bass_exec_p.bind(
            *operands,
            out_avals=tuple(out_avals),
            in_names=tuple(all_names),
            out_names=tuple(out_names),
            lowering_input_output_aliases=(),
            sim_require_finite=True,
            sim_require_nnan=True,
            nc=nc,
        )
        return tuple(outs)

    devices = jax.devices()[:8]
    mesh = Mesh(np.asarray(devices), ("core",))
    in_specs = (PartitionSpec("core"),) * (n_params + n_outs)
    out_specs = (PartitionSpec("core"),) * n_outs
    donate = tuple(range(n_params, n_params + n_outs))
    sharded = jax.jit(
        shard_map(_body, mesh=mesh, in_specs=in_specs, out_specs=out_specs,
                  check_rep=False),
        donate_argnums=donate, keep_unused=True,
    )

    def run(in_maps):
        concat_in = [
            np.concatenate([np.asarray(in_maps[c][nm]) for c in range(8)],
                           axis=0)
            for nm in in_names
        ]
        concat_zeros = [
            np.zeros((8 * s[0], *s[1:]), dt) for (s, dt) in zero_shapes
        ]
        out_arrs = sharded(*concat_in, *concat_zeros)
        return [
            {nm: np.asarray(out_arrs[i]).reshape(8, *out_avals[i].shape)[c]
             for i, nm in enumerate(out_names)}
            for c in range(8)
        ]

    _CACHED["runner"] = run
    return run


def kernel(x, w_attn, b_attn, w_proj, b_proj):
    x = np.asarray(x, dtype=np.float32)
    w_attn = np.asarray(w_attn, dtype=np.float32)
    b_attn = np.asarray(b_attn, dtype=np.float32)
    w_proj = np.asarray(w_proj, dtype=np.float32)
    b_proj = np.asarray(b_proj, dtype=np.float32)

    in_maps = make_in_maps(x, w_attn, b_attn, w_proj)
    try:
        run = _get_runner()
        results = run(in_maps)
    except Exception:
        if "nc" not in _CACHED:
            _CACHED["nc"] = build_nc()
        res = run_bass_kernel_spmd(
            _CACHED["nc"], in_maps, core_ids=list(range(8)))
        results = res.results

    # probs rows sum to 1, so attn += 1 * b_v^T contributes b_v @ w_proj
    # to every row; q.bk terms are softmax-invariant (k bias dropped).
    extra = b_attn[2 * C:] @ w_proj + b_proj  # [C]
    out = np.empty((B, T, C), dtype=np.float32)
    for b in range(B):
        out[b] = (results[2 * b]["y"].astype(np.float32)
                  + results[2 * b + 1]["y"].astype(np.float32) + extra)
    return out


# revision 70
# speedup vs baseline: 1.0050x; 1.0050x over previous
"""Causal multi-head attention (B=4, T=2048, C=1024, H=16) on 8 Trainium2 cores.

Sharding: core c handles batch b = c//2 and heads h0..h0+7 with h0 = (c%2)*8.
Each core computes QKV projection for its head slice, causal attention for its
8 heads, and a partial output projection. Host sums the two partials per batch
and adds the bias terms.

Row-blocked mixed precision (host-validated to ~8e-3 max-rel, gate 2e-2):
 - Query rows 0-511 ("strip 0") run bf16: early rows attend to few keys so
   attn ~ a single v row with sigma ~1, leaving them ~30x less absolute-error
   headroom than late rows (sigma ~0.1).
 - Rows 512-2047 (strips 1-3) run fp8: QKV projection and output projection
   as float8e4 DoubleRow matmuls (0.5 PE cycles/row, 256-deep contraction),
   and AV as DoubleRow over k-tile pairs. Softmax probs u are fp8e5m2,
   written by BOTH engines per k-tile pair: ScalarE native exp for one head
   half (and all diagonal tiles - masked -1e30 scores are unsafe through the
   u8 bitcast), DVE via a Schraudolph exp (u8 = round(A*s + B) bitcast as
   e5m2) for the other, so the scores PSUM ring drains at two-engine rate.
 - AV runs TRANSPOSED: u (stationary, fp8, M=128 q-slice) x v (moving, 65
   cols incl. the ones column) -> attn^T [q, hd] chunks accumulated per
   q-subtile. The softmax rowsum lands in column 64 (per-partition), so
   normalization is reciprocal + one tensor_scalar per q-subtile - no
   cross-partition broadcast. A PE transpose (bf16) then restores the [c, t]
   layout needed by the projection. DoubleRow ldweights requires stationary
   free dim 128/256 (65-wide stationary is rejected by the ISA), which this
   layout sidesteps.
 - Weights host-prescaled x32 (fp8 subnormal avoidance); ones column 4.0 so
   attnT holds 8*attn; the y copy scales by 1/256. k's bias is dropped
   (softmax-invariant); q keeps its bias. A Delta=2 shift inside exp
   (softmax-invariant) keeps exp(s) in e4m3 range.

Scores stay bf16 ([k, q] layout; causal mask as a -1e30 bf16 ident-matmul
into PSUM before the exp; per-128-column causal trimming). Strip-0's
attention is interleaved into the fp8 projection strips (complementary
engine profiles); the output projection merges both column-halves of each
row-block into one 2-bank PSUM tile to halve the copy/DMA count.
"""

import sys
import numpy as np

sys.path.insert(0, "/opt/trn_rl_repo")

import concourse.bass as bass  # noqa: E402
import concourse.bacc as bacc  # noqa: E402
import concourse.mybir as mybir  # noqa: E402
from concourse.bass_utils import run_bass_kernel_spmd  # noqa: E402
from concourse.tile import TileContext  # noqa: E402

B, T, C, H = 4, 2048, 1024, 16
HD = C // H          # 64 head dim
HPC = 8              # heads per core
P = 128
NT = T // P          # 16 t-chunks of 128
NS = T // 512        # 4 q-strips of 512
KC = C // P          # 8 contraction chunks for QKV
CL = HPC * HD        # 512 local channels per section
EH = HD + 1          # 65: head slot width in v (value cols + ones col)
T0 = 512             # bf16/fp8 row boundary
F32 = mybir.dt.float32
BF16 = mybir.dt.bfloat16
E4 = mybir.dt.float8e4
E5 = mybir.dt.float8e5
U8 = mybir.dt.uint8
EXPF = mybir.ActivationFunctionType.Exp
IDENT = mybir.ActivationFunctionType.Identity
MUL = mybir.AluOpType.mult
ADD = mybir.AluOpType.add
DIV = mybir.AluOpType.divide
DRM = mybir.MatmulPerfMode.DoubleRow

WSC = 32.0           # host weight prescale
ONESV = 4.0          # ones-column value -> attnT = 8*attn
YSC = 1.0 / 256.0    # final y scale (1/(8*32))
SC = 0.125 / (WSC * WSC)   # exp scale on raw PSUM scores
DELTA = 2.0          # softmax-invariant shift inside exp
SCH_A = float((4.0 / np.log(2.0)) * SC)
SCH_B = float(4.0 * 15 - 0.23 - (4.0 / np.log(2.0)) * DELTA)

_CACHED = {}


def build_nc():
    nc = bacc.Bacc("TRN2", target_bir_lowering=False, debug=False)

    xtb_d = nc.dram_tensor("xtb", [C, T0], BF16, kind="ExternalInput")
    xt8_d = nc.dram_tensor("xt8", [C, T - T0], E4, kind="ExternalInput")
    wqkb_d = nc.dram_tensor("wqkb", [C, 2 * CL], BF16, kind="ExternalInput")
    wqk8_d = nc.dram_tensor("wqk8", [C, 2 * CL], E4, kind="ExternalInput")
    wvb_d = nc.dram_tensor("wvb", [C, CL], BF16, kind="ExternalInput")
    wv8_d = nc.dram_tensor("wv8", [C, CL], E4, kind="ExternalInput")
    wpb_d = nc.dram_tensor("wpb", [CL, C], BF16, kind="ExternalInput")
    wp8_d = nc.dram_tensor("wp8", [CL, C], E4, kind="ExternalInput")
    bq_d = nc.dram_tensor("bq", [P, 4], F32, kind="ExternalInput")
    ident_d = nc.dram_tensor("ident", [P, P], BF16, kind="ExternalInput")
    maskb_d = nc.dram_tensor("maskb", [P, P], BF16, kind="ExternalInput")
    y_d = nc.dram_tensor("y", [T, C], BF16, kind="ExternalOutput")

    xtb_r = xtb_d.ap().rearrange("(kc p) t -> p kc t", p=P)     # [128,8,512]
    xt8_r = xt8_d.ap().rearrange("(kc p) t -> p kc t", p=P)     # [128,8,1536]
    wqkb_r = wqkb_d.ap().rearrange("(kc p) c -> p kc c", p=P)   # [128,8,1024]
    wqk8_r = wqk8_d.ap().rearrange("(kc p) c -> p kc c", p=P)
    wvb_r = wvb_d.ap().rearrange("(kc p) c -> p kc c", p=P)     # [128,8,512]
    wv8_r = wv8_d.ap().rearrange("(kc p) c -> p kc c", p=P)
    wpb_r = wpb_d.ap().rearrange("(ct p) c -> p ct c", p=P)     # [128,4,1024]
    wp8_r = wp8_d.ap().rearrange("(ct p) c -> p ct c", p=P)
    y_r = y_d.ap().rearrange("(tt p) c -> p tt c", p=P)         # [128,16,1024]

    SW = 256             # phase-A t-strip width
    NB = T0 // SW        # 2 bf16 strips
    NTB = T0 // P        # 4 bf16 t-chunks

    with TileContext(nc) as tc:
      with tc.tile_pool(name="const", bufs=1) as constp:
        ident = constp.tile([P, P], BF16)
        maskb = constp.tile([P, P], BF16)
        bq = constp.tile([P, 4], F32)
        bias_m2 = constp.tile([P, 1], F32)
        nc.vector.memset(bias_m2[:], -DELTA)

        with tc.tile_pool(name="big", bufs=1) as bigp:
            qkT = bigp.tile([P, 8, T], BF16)  # c-tiles 0-3 qT (biased), 4-7 kT
            v_bf = bigp.tile([P, NTB, HPC * EH], BF16)
            v_f8 = bigp.tile([P, NT, HPC * EH], E4)
            vbf_h = v_bf[:].rearrange("p t (h e) -> p t h e", e=EH)
            vf8_h = v_f8[:].rearrange("p t (h e) -> p t h e", e=EH)
            nc.gpsimd.memset(vbf_h[:, :, :, HD], ONESV)
            nc.gpsimd.memset(vf8_h[:, :, :, HD], ONESV)
            attnT_bf = bigp.tile([P, 4, T0], BF16)
            attnT_f8 = bigp.tile([P, 4, T - T0], E4)
            wpb_sb = bigp.tile([P, 4, C], BF16)
            wp8_sb = bigp.tile([P, 4, C], E4)

            # ------- Phase A (QKV) with strip-0 attention overlapped -------
            # The fp8 projection strips (ts 2-7) are PE/DMA work with little
            # Act/DVE load, while strip-0 attention is exp/copy-heavy with
            # little PE work: interleaving their emission fills both. Strip-0
            # only needs ts 0-1 (q,k,v rows < 512), so its pairs slot in
            # between the later projection strips. PSUM during the overlap:
            # psa 1 + psq 2 + ring 2 + po 2 + psT 1 = 8 banks.
            with (
                tc.tile_pool(name="u_act", bufs=16) as uap,
                tc.tile_pool(name="u_dve", bufs=12) as udp,
                tc.tile_pool(name="u_bf", bufs=10) as u0p,
                tc.tile_pool(name="an_p", bufs=10) as anp,
            ):
                expctr = [0]

                def emit_scores_exp(qj, pr, ps_s):
                    """Scores + exp for one (strip, pair). Returns the list
                    of u tiles (one per kt-pair for qj>0, per kt for qj==0)
                    for the AV stage."""
                    qct, kct = pr, 4 + pr
                    qlo, qhi = qj * 512, (qj + 1) * 512
                    us = []
                    if qj == 0:
                        for kt in range(4):
                            q0 = kt * P
                            u = u0p.tile([P, 2, 512], BF16, tag="u0")
                            for hh in range(2):
                                hp = hh * HD
                                ps = ps_s.tile([P, 512], F32, tag="ps")
                                nc.tensor.matmul(
                                    ps[:, q0:512],
                                    qkT[hp:hp + HD, kct,
                                        kt * P:(kt + 1) * P],
                                    qkT[hp:hp + HD, qct, q0:512],
                                    start=True, stop=False,
                                )
                                nc.tensor.matmul(
                                    ps[:, q0:q0 + P], ident[:], maskb[:],
                                    start=False, stop=True,
                                )
                                nc.scalar.activation(
                                    u[:, hh, q0:512], ps[:, q0:512],
                                    EXPF, scale=SC, bias=bias_m2[:])
                            us.append(u)
                        return us
                    npair = 2 * (qj + 1)
                    for ktp in range(npair):
                        ktA, ktB = 2 * ktp, 2 * ktp + 1
                        diag = ktA >= 4 * qj
                        q0A = max(0, ktA * P - qj * 512)
                        q0B = max(0, ktB * P - qj * 512)
                        # each ktp's 4 exp units split across BOTH engines
                        # (hh0 -> Act native exp, hh1 -> DVE schraudolph,
                        # both writing e5m2) so neither engine idles while
                        # the other works. Diagonal tiles stay all-Act
                        # (-1e30 masked scores are unsafe through the u8
                        # bitcast).
                        u = uap.tile([P, 2, 2, 512], E5, tag="u5a")
                        for kb, (kt, q0) in enumerate(
                                ((ktA, q0A), (ktB, q0B))):
                            for hh in range(2):
                                hp = hh * HD
                                ps = ps_s.tile([P, 512], F32, tag="ps")
                                nc.tensor.matmul(
                                    ps[:, q0:512],
                                    qkT[hp:hp + HD, kct,
                                        kt * P:(kt + 1) * P],
                                    qkT[hp:hp + HD, qct,
                                        qlo + q0:qhi],
                                    start=True, stop=not diag,
                                )
                                if diag:
                                    nc.tensor.matmul(
                                        ps[:, q0:q0 + P], ident[:], maskb[:],
                                        start=False, stop=True,
                                    )
                                if diag or hh == 0:
                                    nc.scalar.activation(
                                        u[:, kb, hh, q0:512], ps[:, q0:512],
                                        EXPF, scale=SC, bias=bias_m2[:])
                                else:
                                    nc.vector.tensor_scalar(
                                        u[:, kb, hh, q0:512].bitcast(U8),
                                        ps[:, q0:512],
                                        SCH_A, SCH_B, MUL, ADD)
                        if diag and q0B > q0A:
                            # block B's columns [q0A:q0B] are fully masked
                            # but get read by the q0A-subtile AV chain
                            nc.gpsimd.memset(u[:, 1, :, q0A:q0B], 0.0)
                        if not diag:
                            expctr[0] += 1
                        us.append(u)
                    return us

                def emit_av_norm(qj, pr, us, ps_o, ps_t):
                    """Transposed AV + normalize + transpose into attnT."""
                    psT = ps_t.tile([P, 1024], BF16, tag="psT")
                    for qt in range(4):
                        poQ = {}
                        for hh in range(2):
                            h = 2 * pr + hh
                            po = ps_o.tile([P, 512], F32, tag="po")
                            poQ[hh] = po
                            if qj == 0:
                                chain = [(kt, us[kt]) for kt in range(qt + 1)]
                                for i, (kt, u) in enumerate(chain):
                                    nc.tensor.matmul(
                                        po[:, 0:EH],
                                        u[:, hh, qt * P:(qt + 1) * P],
                                        vbf_h[:, kt, h, :],
                                        start=(i == 0),
                                        stop=(i == len(chain) - 1),
                                    )
                            else:
                                chain = []
                                for ktp, u in enumerate(us):
                                    q0A = max(0, 2 * ktp * P - qj * 512)
                                    if q0A <= qt * P:
                                        chain.append((ktp, u))
                                for i, (ktp, u) in enumerate(chain):
                                    nc.tensor.matmul(
                                        po[:, 0:EH],
                                        u[:, :, hh, qt * P:(qt + 1) * P],
                                        vf8_h[:, 2 * ktp:2 * ktp + 2, h, :],
                                        start=(i == 0),
                                        stop=(i == len(chain) - 1),
                                        perf_mode=DRM,
                                    )
                        for hh in range(2):
                            po = poQ[hh]
                            rcol = anp.tile([P, 1], F32, tag="rc")
                            nc.vector.reciprocal(rcol[:], po[:, HD:EH])
                            a_n = anp.tile([P, HD], BF16, tag="an")
                            nc.vector.tensor_scalar(
                                a_n[:], po[:, 0:HD], rcol[:], None, MUL)
                            # 8 transposes share one PSUM bank: open the
                            # zero region once per partition half (qt==0),
                            # close on the last qt - a fresh start=True per
                            # transpose would re-mark the whole 2KB region
                            # pending-zero and wipe the sibling writes.
                            nc.tensor.matmul(
                                psT[hh * HD:(hh + 1) * HD,
                                    qt * P:(qt + 1) * P],
                                a_n[:], ident[:], is_transpose=True,
                                start=(qt == 0), stop=(qt == 3),
                            )
                    if qj == 0:
                        nc.scalar.copy(
                            attnT_bf[:, pr, qj * 512:(qj + 1) * 512],
                            psT[:, 0:512])
                    else:
                        nc.vector.tensor_copy(
                            attnT_f8[:, pr, qj * 512 - T0:(qj + 1) * 512 - T0],
                            psT[:, 0:512])

                with (
                    tc.tile_pool(name="xts", bufs=4) as xtsp,
                    tc.tile_pool(name="w_pool", bufs=1) as wqp,
                    tc.tile_pool(name="ps_av", bufs=1, space="PSUM") as ps_av,
                    tc.tile_pool(name="ps_aq", bufs=2, space="PSUM") as ps_aq,
                    tc.tile_pool(name="ps_s1", bufs=2, space="PSUM") as ps_s1,
                    tc.tile_pool(name="ps_o1", bufs=2, space="PSUM") as ps_o1,
                    tc.tile_pool(name="ps_t1", bufs=1, space="PSUM") as ps_t1,
                ):
                    wvb_sb = wqp.tile([P, KC, CL], BF16)
                    wv8_sb = wqp.tile([P, KC, CL], E4)
                    wqkb_sb = wqp.tile([P, KC, 2 * CL], BF16)
                    wqk8_sb = wqp.tile([P, KC, 2 * CL], E4)
                    xts0 = xtsp.tile([P, KC, SW], BF16, tag="xtsb")
                    nc.sync.dma_start(xts0[:, 0, :], xtb_r[:, 0, 0:SW])
                    nc.sync.dma_start(wvb_sb[:, 0, :], wvb_r[:, 0, :])
                    nc.sync.dma_start(ident[:], ident_d[:])
                    nc.sync.dma_start(maskb[:], maskb_d[:])
                    nc.sync.dma_start(bq[:], bq_d[:])
                    nc.sync.dma_start(xts0[:, 1:KC, :], xtb_r[:, 1:KC, 0:SW])
                    nc.sync.dma_start(wvb_sb[:, 1:KC, :], wvb_r[:, 1:KC, :])
                    nc.sync.dma_start(wqkb_sb[:], wqkb_r)

                    # prefetch-next x-strip: strip ts+1's DMA is issued ahead
                    # of deferred weight loads so it never queues behind them
                    def xts_dma(ts):
                        if ts >= T // SW:
                            return None
                        if ts < NB:
                            t = xtsp.tile([P, KC, SW], BF16, tag="xtsb",
                                          name=f"xts_{ts}")
                            nc.sync.dma_start(
                                t[:], xtb_r[:, :, ts * SW:(ts + 1) * SW])
                        else:
                            t = xtsp.tile([P, KC, SW], E4, tag="xts8",
                                          name=f"xts_{ts}")
                            nc.sync.dma_start(
                                t[:],
                                xt8_r[:, :, ts * SW - T0:(ts + 1) * SW - T0])
                        return t

                    pf = {1: xts_dma(1)}
                    nc.sync.dma_start(wv8_sb[:], wv8_r)
                    nc.sync.dma_start(wqk8_sb[:], wqk8_r)

                    def emit_ts(ts):
                        is_bf = ts < NB
                        xts = xts0 if ts == 0 else pf.pop(ts)
                        pf[ts + 1] = xts_dma(ts + 1)
                        if ts == 5:
                            # proj weights aren't needed until phase C
                            # (~170us); keep them out of the DMA stream
                            # while the input tensors are still draining
                            nc.sync.dma_start(wpb_sb[:], wpb_r)
                            nc.sync.dma_start(wp8_sb[:], wp8_r)
                        # v part: [t, c] orientation
                        for tt in range(SW // P):
                            tch = ts * (SW // P) + tt
                            psv = ps_av.tile([P, CL], F32, tag="psa")
                            if is_bf:
                                for kc in range(KC):
                                    nc.tensor.matmul(
                                        psv[:],
                                        xts[:, kc, tt * P:(tt + 1) * P],
                                        wvb_sb[:, kc, :],
                                        start=(kc == 0), stop=(kc == KC - 1),
                                    )
                                nc.vector.tensor_copy(
                                    vbf_h[:, tch, :, 0:HD],
                                    psv[:].rearrange("p (h d) -> p h d", d=HD),
                                )
                            else:
                                for kc2 in range(KC // 2):
                                    nc.tensor.matmul(
                                        psv[:],
                                        xts[:, 2 * kc2:2 * kc2 + 2,
                                            tt * P:(tt + 1) * P],
                                        wv8_sb[:, 2 * kc2:2 * kc2 + 2, :],
                                        start=(kc2 == 0),
                                        stop=(kc2 == KC // 2 - 1),
                                        perf_mode=DRM,
                                    )
                            nc.vector.tensor_copy(
                                vf8_h[:, tch, :, 0:HD],
                                psv[:].rearrange("p (h d) -> p h d", d=HD),
                            )
                        # qT/kT part: [c, t] orientation; bf16 strips emit
                        # c-tiles in head-pair order (q0,k0,q1,k1,...) so
                        # strip-0 pair pr can score as soon as its two
                        # tiles land
                        cts = [0, 4, 1, 5, 2, 6, 3, 7] if is_bf else range(8)
                        for ct in cts:
                            psq = ps_aq.tile([P, SW], F32, tag="psq")
                            if is_bf:
                                for kc in range(KC):
                                    nc.tensor.matmul(
                                        psq[:],
                                        wqkb_sb[:, kc, ct * P:(ct + 1) * P],
                                        xts[:, kc, :],
                                        start=(kc == 0), stop=(kc == KC - 1),
                                    )
                            else:
                                for kc2 in range(KC // 2):
                                    nc.tensor.matmul(
                                        psq[:],
                                        wqk8_sb[:, 2 * kc2:2 * kc2 + 2,
                                                ct * P:(ct + 1) * P],
                                        xts[:, 2 * kc2:2 * kc2 + 2, :],
                                        start=(kc2 == 0),
                                        stop=(kc2 == KC // 2 - 1),
                                        perf_mode=DRM,
                                    )
                            dst = qkT[:, ct, ts * SW:(ts + 1) * SW]
                            if ct < 4 and ct % 2 == 0:
                                nc.scalar.activation(
                                    dst, psq[:], IDENT, bias=bq[:, ct:ct + 1])
                            elif ct < 4:
                                nc.vector.tensor_scalar(
                                    dst, psq[:], bq[:, ct:ct + 1], None, ADD)
                            elif ct % 2 == 0:
                                nc.vector.tensor_copy(dst, psq[:])
                            else:
                                nc.scalar.copy(dst, psq[:])

                    emit_ts(0)
                    emit_ts(1)
                    s0us = []
                    s0us.append(emit_scores_exp(0, 0, ps_s1))
                    emit_ts(2)
                    emit_ts(3)
                    s0us.append(emit_scores_exp(0, 1, ps_s1))
                    emit_av_norm(0, 0, s0us[0], ps_o1, ps_t1)
                    emit_ts(4)
                    s0us.append(emit_scores_exp(0, 2, ps_s1))
                    emit_av_norm(0, 1, s0us[1], ps_o1, ps_t1)
                    emit_ts(5)
                    s0us.append(emit_scores_exp(0, 3, ps_s1))
                    emit_av_norm(0, 2, s0us[2], ps_o1, ps_t1)
                    emit_ts(6)
                    # strip-1 pr0's scores ride the overlap ring too, so the
                    # big-ring scope below starts with an AV already fed
                    s1u0 = emit_scores_exp(1, 0, ps_s1)
                    emit_ts(7)
                    emit_av_norm(0, 3, s0us[3], ps_o1, ps_t1)

                # ---- strips 1-3: full-depth scores ring ----
                with (
                    tc.tile_pool(name="ps_s2", bufs=5, space="PSUM") as ps_s2,
                    tc.tile_pool(name="ps_o2", bufs=2, space="PSUM") as ps_o2,
                    tc.tile_pool(name="ps_t2", bufs=1, space="PSUM") as ps_t2,
                ):
                    prev = (1, 0, s1u0, ps_o2, ps_t2)
                    for qj in range(1, NS):
                        for pr in range(4):
                            if qj == 1 and pr == 0:
                                continue
                            us = emit_scores_exp(qj, pr, ps_s2)
                            if prev is not None:
                                emit_av_norm(*prev)
                            prev = (qj, pr, us, ps_o2, ps_t2)
                    if prev is not None:
                        emit_av_norm(*prev)

            # ---------------- Phase C: output projection ----------------
            with (
                tc.tile_pool(name="ystage", bufs=8) as ystagep,
                tc.tile_pool(name="ps_f", bufs=4, space="PSUM") as ps_f,
            ):
                i = 0
                # fp8 chains (tt>=4) are 4x shorter on the PE than the bf16
                # ones; lead with a few so the copy/DMA drain starts early.
                # Both co-halves of a tt share one 2-bank psy tile (halves
                # are 2KB-aligned, so their accumulation groups don't clash)
                # -> one [128,1024] copy + DMA per tt instead of two.
                ttorder = [4, 5, 0, 6, 7, 1, 8, 9, 2, 10, 11, 3,
                           12, 13, 14, 15]
                for tt in ttorder:
                    psy = ps_f.tile([P, 1024], F32, tag="psf",
                                    name=f"psf_{tt}")
                    for co in range(2):
                        if tt < 4:
                            for ct in range(4):
                                nc.tensor.matmul(
                                    psy[:, co * 512:(co + 1) * 512],
                                    attnT_bf[:, ct, tt * P:(tt + 1) * P],
                                    wpb_sb[:, ct, co * 512:(co + 1) * 512],
                                    start=(ct == 0), stop=(ct == 3),
                                )
                        else:
                            for cp in range(2):
                                nc.tensor.matmul(
                                    psy[:, co * 512:(co + 1) * 512],
                                    attnT_f8[:, 2 * cp:2 * cp + 2,
                                             (tt - 4) * P:(tt - 3) * P],
                                    wp8_sb[:, 2 * cp:2 * cp + 2,
                                           co * 512:(co + 1) * 512],
                                    start=(cp == 0), stop=(cp == 1),
                                    perf_mode=DRM,
                                )
                    yt = ystagep.tile([P, 1024], BF16, tag="yt")
                    if i % 2 == 0:
                        nc.scalar.mul(yt[:], psy[:], YSC)
                    else:
                        nc.vector.tensor_scalar(
                            yt[:], psy[:], YSC, None, MUL)
                    nc.sync.dma_start(y_r[:, tt, :], yt[:])
                    i += 1
    nc.compile()
    return nc


def _host_consts():
    import ml_dtypes
    i_idx = np.arange(P, dtype=np.float32)[:, None]
    j_idx = np.arange(P, dtype=np.float32)[None, :]
    maskb = np.where(j_idx - i_idx >= 0, 0.0, -1e30).astype(ml_dtypes.bfloat16)
    ident = np.eye(P, dtype=ml_dtypes.bfloat16)
    return ident, maskb


def make_in_maps(x, w_attn, b_attn, w_proj):
    import ml_dtypes
    ident, maskb = _host_consts()
    in_maps = []
    for c in range(8):
        b = c // 2
        h0 = (c % 2) * HPC
        qcols = slice(h0 * HD, h0 * HD + CL)
        kcols = slice(C + h0 * HD, C + h0 * HD + CL)
        vcols = slice(2 * C + h0 * HD, 2 * C + h0 * HD + CL)
        wqk = np.concatenate([w_attn[:, qcols], w_attn[:, kcols]],
                             axis=1) * WSC
        wv = w_attn[:, vcols] * WSC
        wp = w_proj[h0 * HD:h0 * HD + CL, :] * WSC
        bqv = (b_attn[qcols] * WSC).reshape(4, P).T  # [128, 4] q bias only
        xt = np.ascontiguousarray(x[b].T)
        in_maps.append({
            "xtb": xt[:, :T0].astype(ml_dtypes.bfloat16),
            "xt8": xt[:, T0:].astype(ml_dtypes.float8_e4m3),
            "wqkb": wqk.astype(ml_dtypes.bfloat16),
            "wqk8": wqk.astype(ml_dtypes.float8_e4m3),
            "wvb": wv.astype(ml_dtypes.bfloat16),
            "wv8": wv.astype(ml_dtypes.float8_e4m3),
            "wpb": wp.astype(ml_dtypes.bfloat16),
            "wp8": wp.astype(ml_dtypes.float8_e4m3),
            "bq": np.ascontiguousarray(bqv).astype(np.float32),
            "ident": ident,
            "maskb": maskb,
        })
    return in_maps


def _get_runner():
    """Build the SPMD executor once: a cached jax.jit over 8 cores."""
    if "runner" in _CACHED:
        return _CACHED["runner"]
    import jax
    from jax.sharding import Mesh, PartitionSpec
    from jax.experimental.shard_map import shard_map
    from concourse import bass2jax
    import concourse.mybir as mybir_

    nc = _CACHED.get("nc")
    if nc is None:
        nc = _CACHED["nc"] = build_nc()
    bass2jax.install_neuronx_cc_hook()

    partition_name = (nc.partition_id_tensor.name
                      if nc.partition_id_tensor else None)
    in_names, out_names, out_avals, zero_shapes = [], [], [], []
    for alloc in nc.m.functions[0].allocations:
        if not isinstance(alloc, mybir_.MemoryLocationSet):
            continue
        name = alloc.memorylocations[0].name
        if alloc.kind == "ExternalInput":
            if name != partition_name:
                in_names.append(name)
        elif alloc.kind == "ExternalOutput":
            shape = tuple(alloc.tensor_shape)
            dtype = mybir_.dt.np(alloc.dtype)
            out_names.append(name)
            out_avals.append(jax.core.ShapedArray(shape, dtype))
            zero_shapes.append((shape, dtype))
    n_params = len(in_names)
    n_outs = len(out_names)
    all_names = in_names + out_names
    if partition_name is not None:
        all_names = all_names + [partition_name]

    def _body(*args):
        operands = list(args)
        if partition_name is not None:
            operands.append(bass2jax.partition_id_tensor())
        outs = bass2jax._bass_exec_p.bind(
            *operands,
            out_avals=tuple(out_avals),
            in_names=tuple(all_names),
            out_names=tuple(out_names),
            lowering_input_output_aliases=(),
            sim_require_finite=True,
            sim_require_nnan=True,
            nc=nc,
        )
        return tuple(outs)

    devices = jax.devices()[:8]
    mesh = Mesh(np.asarray(devices), ("core",))
    in_specs = (PartitionSpec("core"),) * (n_params + n_outs)
    out_specs = (PartitionSpec("core"),) * n_outs
    donate = tuple(range(n_params, n_params + n_outs))
    sharded = jax.jit(
        shard_map(_body, mesh=mesh, in_specs=in_specs, out_specs=out_specs,
                  check_rep=False),
        donate_argnums=donate, keep_unused=True,
    )

    def run(in_maps):
        concat_in = [
            np.concatenate([np.asarray(in_maps[c][nm]) for c in range(8)],
                           axis=0)
            for nm in in_names
        ]
        concat_zeros = [
            np.zeros((8 * s[0], *s[1:]), dt) for (s, dt) in zero_shapes
        ]
        out_arrs = sharded(*concat_in, *concat_zeros)
        return [
            {nm: np.asarray(out_arrs[i]).reshape(8, *out_avals[i].shape)[c]
             for i, nm in enumerate(out_names)}
            for c in range(8)
        ]

    _CACHED["runner"] = run
    return run


def kernel(x, w_attn, b_attn, w_proj, b_proj):
    x = np.asarray(x, dtype=np.float32)
    w_attn = np.asarray(w_attn, dtype=np.float32)
    b_attn = np.asarray(b_attn, dtype=np.float32)
    w_proj = np.asarray(w_proj, dtype=np.float32)
    b_proj = np.asarray(b_proj, dtype=np.float32)

    in_maps = make_in_maps(x, w_attn, b_attn, w_proj)
    try:
        run = _get_runner()
        results = run(in_maps)
    except Exception:
        if "nc" not in _CACHED:
            _CACHED["nc"] = build_nc()
        res = run_bass_kernel_spmd(
            _CACHED["nc"], in_maps, core_ids=list(range(8)))
        results = res.results

    # probs rows sum to 1, so attn += 1 * b_v^T contributes b_v @ w_proj
    # to every row; q.bk terms are softmax-invariant (k bias dropped).
    extra = b_attn[2 * C:] @ w_proj + b_proj  # [C]
    out = np.empty((B, T, C), dtype=np.float32)
    for b in range(B):
        out[b] = (results[2 * b]["y"].astype(np.float32)
                  + results[2 * b + 1]["y"].astype(np.float32) + extra)
    return out
